# revision 13
# baseline (speedup 1.0000x reference)
"""FBCritic embedding-lookup kernel for 8 Trainium2 NeuronCores.

Math (reference):
    fwd_idx = clip(obs)*10 + clip(act)            # [8192]
    bwd_idx = clip(fobs)*10 + clip(fact)          # [8192]
    F = W_f[fwd_idx]                              # [8192, 64]
    B = W_b[bwd_idx]                              # [8192, 64]
    out = F @ B.T                                 # [8192, 8192] f32

Sharding: 2D grid over the output — 4 row blocks x 2 col blocks. Core
c = a*2 + b computes out[a*2048:(a+1)*2048, b*4096:(b+1)*4096]. The 2D
split minimizes indirect-gather instructions per core (16 fwd + 32 bwd
128-row groups): the SWDGE consumes exactly one index per destination
partition per indirect DMA and costs ~1us of GpSimd per instruction,
serialized, making gather count the second roofline next to the output
DMA bytes (16 MiB f16 per core).

Precision: tables are converted to f16 on the host (output rel err
~3e-4 vs the 2e-2 gate), the device computes in f16 with f32 PSUM
accumulation, and the output is written f16 and upcast on the host.

Pipeline: gathers are issued in "quads" (4 x 128-row groups = one
512-wide column chunk for the backward side, or 4 row tiles for the
forward side) following a ladder that interleaves forward and backward
quads so output strips (one per row-tile x column-chunk) unlock at a
steady rate for the DMA pipe. The two final backward chunks are only
2 groups wide so the post-gather drain is short. PE transposes each
gathered [128, 64] f16 group via identity matmul into f16 PSUM; DVE
assembles the [64, n] operands (f16 2x mode); matmuls accumulate f32
PSUM strips; PSUM->SBUF strip copies alternate strictly DVE/ACT (strict
alternation keeps either engine from head-blocking on a future chunk);
output DMAs go out on the sync (SP) HWDGE queue.
"""

import numpy as np

NUM_OBS = 100000
NUM_ACT = 10
V = NUM_OBS * NUM_ACT  # 1_000_000 table rows
D = 64                 # repr dim
B = 8192               # batch
N_CORES = 8
RA = 4                 # row blocks
CB = 2                 # col blocks
MLOC = B // RA         # 2048 output rows per core
NLOC = B // CB         # 4096 output cols per core
P = 128                # partitions

GF = MLOC // P         # 16 forward 128-row groups
GB = NLOC // P         # 32 backward 128-row groups

# Backward column chunks in 128-col groups. 1024-col chunks (output
# strips are 1024 wide: the HWDGE costs ~625ns per output DMA, capping
# useful DMA count), with two 512-col chunks at the end for a short
# post-gather tail.
CHUNK_GROUPS = [8, 8, 8, 8]
assert sum(CHUNK_GROUPS) == GB

_CACHE = {}


def _build_nc():
    import concourse.bass as bass
    import concourse.tile as tile
    from concourse import bacc, mybir
    from concourse.masks import make_identity

    f16 = mybir.dt.float16
    f32 = mybir.dt.float32
    i32 = mybir.dt.int32

    nc = bacc.Bacc("TRN2", target_bir_lowering=False, debug=False)

    wf = nc.dram_tensor("wf", [V, D], f16, kind="ExternalInput").ap()
    wb = nc.dram_tensor("wb", [V, D], f16, kind="ExternalInput").ap()
    idxf_d = nc.dram_tensor("idxf", [P, GF], i32, kind="ExternalInput").ap()
    idxb_d = nc.dram_tensor("idxb", [P, GB], i32, kind="ExternalInput").ap()
    out_d = nc.dram_tensor("out", [MLOC, NLOC], f16, kind="ExternalOutput").ap()

    n_copy = [0]
    STRIP_PAT = [1, 0, 1, 0, 1, 0, 1, 1, 0, 1, 0, 1, 0, 1, 1, 0]  # 1 = ACT (9/16)

    with tile.TileContext(nc) as tc:
        with (
            tc.tile_pool(name="const", bufs=1) as const_pool,
            tc.tile_pool(name="idx", bufs=1) as idx_pool,
            tc.tile_pool(name="bg", bufs=16) as bg_pool,
            tc.tile_pool(name="ops", bufs=1) as ops_pool,
            tc.tile_pool(name="strip", bufs=8) as strip_pool,
            tc.tile_pool(name="tpsum", bufs=2, space="PSUM") as tpsum_pool,
            tc.tile_pool(name="mpsum", bufs=3, space="PSUM") as mpsum_pool,
        ):
            identity = const_pool.tile([P, P], f16)
            make_identity(nc, identity[:])

            idxf = idx_pool.tile([P, GF], i32, tag="idxf")
            idxb = idx_pool.tile([P, GB], i32, tag="idxb")
            nc.sync.dma_start(idxf[:], idxf_d[:])
            nc.sync.dma_start(idxb[:], idxb_d[:])

            fwdT = ops_pool.tile([D, MLOC], f16, tag="fwdT")
            bwdT = ops_pool.tile([D, NLOC], f16, tag="bwdT")

            def gat_unit(table, idx_tile, g0, n, dstT, d0):
                """Gather n groups (g0..g0+n-1), PE-transpose, copy into
                dstT[:, d0:d0+n*128]. Processes in sub-units of <=4 groups
                (one [64, 512] f16 PSUM tile each)."""
                for s0 in range(0, n, 4):
                    ns = min(4, n - s0)
                    pt = tpsum_pool.tile([D, 512], f16, tag="pt")
                    for r in range(ns):
                        t = bg_pool.tile([P, D], f16, tag="bg")
                        nc.gpsimd.indirect_dma_start(
                            out=t[:],
                            out_offset=None,
                            in_=table[:],
                            in_offset=bass.IndirectOffsetOnAxis(
                                ap=idx_tile[:, g0 + s0 + r:g0 + s0 + r + 1],
                                axis=0,
                            ),
                        )
                        nc.tensor.transpose(
                            out=pt[:, r * P:(r + 1) * P],
                            in_=t[:],
                            identity=identity[:],
                        )
                    nc.vector.tensor_copy(
                        out=dstT[:, d0 + s0 * P:d0 + (s0 + ns) * P],
                        in_=pt[:, :ns * P],
                    )

            # Column chunk table: (col0, width)
            chunks = []
            c0 = 0
            for g in CHUNK_GROUPS:
                chunks.append((c0, g * P))
                c0 += g * P

            def mm_strip(i, ci):
                col0, w = chunks[ci]
                ps = mpsum_pool.tile([P, 1024], f32, tag="ps")
                for q in range(w // 512):
                    j0 = col0 + q * 512
                    nc.tensor.matmul(
                        out=ps[:, q * 512:(q + 1) * 512],
                        lhsT=fwdT[:, i * P:(i + 1) * P],
                        rhs=bwdT[:, j0:j0 + 512],
                        start=True,
                        stop=True,
                    )
                strip = strip_pool.tile([P, 1024], f16, tag="strip")
                if STRIP_PAT[n_copy[0] % 16]:
                    nc.scalar.copy(out=strip[:, :w], in_=ps[:, :w])
                else:
                    nc.vector.tensor_copy(out=strip[:, :w], in_=ps[:, :w])
                n_copy[0] += 1
                nc.sync.dma_start(
                    out_d[i * P:(i + 1) * P, col0:col0 + w], strip[:, :w]
                )

            # --- Ladder ---------------------------------------------------
            fwd_done = 0
            chunks_done = 0
            emitted = set()

            def emit_ready():
                for ci in range(chunks_done):
                    for i in range(fwd_done):
                        if (i, ci) not in emitted:
                            emitted.add((i, ci))
                            mm_strip(i, ci)

            def fwd_quad(q):
                nonlocal fwd_done
                gat_unit(wf, idxf, 4 * q, 4, fwdT, 512 * q)
                fwd_done += 4

            bwd_g = 0

            def bwd_chunk():
                nonlocal bwd_g, chunks_done
                col0, w = chunks[chunks_done]
                gat_unit(wb, idxb, bwd_g, w // P, bwdT, col0)
                bwd_g += w // P
                chunks_done += 1

            # fwd quads and bwd chunks interleaved for a steady unlock rate.
            for step in ("f0", "b", "f1", "b", "f2", "b", "f3", "b"):
                if step.startswith("f"):
                    fwd_quad(int(step[1]))
                else:
                    bwd_chunk()
                emit_ready()

    nc.compile()
    return nc


def _get_nc():
    if "nc" not in _CACHE:
        _CACHE["nc"] = _build_nc()
    return _CACHE["nc"]


def _ravel_clip(obs, act):
    o = np.clip(obs.astype(np.int64), 0, NUM_OBS - 1)
    a = np.clip(act.astype(np.int64), 0, NUM_ACT - 1)
    return (o * NUM_ACT + a).astype(np.int32)


def make_in_maps(observations, actions, future_observations, future_actions,
                 W_f, W_b):
    fwd_idx = _ravel_clip(np.asarray(observations), np.asarray(actions))
    bwd_idx = _ravel_clip(np.asarray(future_observations),
                          np.asarray(future_actions))
    wf = np.asarray(W_f, dtype=np.float16)
    wb = np.asarray(W_b, dtype=np.float16)
    in_maps = []
    for c in range(N_CORES):
        a, b = divmod(c, CB)
        # [p, g] = idx[g*128 + p]
        idxf = np.ascontiguousarray(
            fwd_idx[a * MLOC:(a + 1) * MLOC].reshape(GF, P).T
        )
        idxb = np.ascontiguousarray(
            bwd_idx[b * NLOC:(b + 1) * NLOC].reshape(GB, P).T
        )
        in_maps.append({"wf": wf, "wb": wb, "idxf": idxf, "idxb": idxb})
    return in_maps


def kernel(**inputs):
    from concourse.bass_utils import run_bass_kernel_spmd

    in_maps = make_in_maps(
        inputs["observations"], inputs["actions"],
        inputs["future_observations"], inputs["future_actions"],
        inputs["W_f"], inputs["W_b"],
    )
    res = run_bass_kernel_spmd(_get_nc(), in_maps, core_ids=list(range(N_CORES)))
    full = np.empty((B, B), dtype=np.float32)
    for c in range(N_CORES):
        a, b = divmod(c, CB)
        full[a * MLOC:(a + 1) * MLOC, b * NLOC:(b + 1) * NLOC] = (
            res.results[c]["out"].astype(np.float32)
        )
    return full
